# revision 14
# baseline (speedup 1.0000x reference)
"""Trainium2 Bass kernel for nn_Decoder (attention + GRU cell + log-softmax LM head).

Sharding over 8 NeuronCores (single SPMD program, per-core input shards):
  - Attention/alignment: data-parallel over batch (8 rows/core).
  - GRU: tensor-parallel over the 3H gate dim (each core owns a 128-wide h slice).
  - LM head: vocab-parallel (each core owns 4096 rows of the zero-padded
    32768-row lsm_W); log-softmax normalizers combined via AllGather.
  - 3 small AllGathers: rnn_in [64,2560], h_newT slices [128,64], stats [64,2].

Matmuls run fp32r (full-rate fp32 PE mode) except the big LM-head matmul,
which runs bf16 (weights cast during the SWDGE DMA load, transposed on PE).
"""

import numpy as np

B, TX, H, E, KY = 64, 128, 1024, 512, 32000
NCORES = 8
BL = B // NCORES          # 8 local batch rows
TWOH = 2 * H              # 2048
K3H = 3 * H               # 3072
KIN = E + TWOH            # 2560
HS = H // NCORES          # 128 gate slice per core
VSH = 4096                # padded vocab shard per core
P = 128
ROWS = TX * BL            # 1024 alignment rows per core; row = b*128 + t

_CACHE = {}


def _program():
    import concourse.bass as bass
    import concourse.mybir as mybir
    import concourse.tile as tile
    from concourse import bacc
    from concourse.masks import make_identity

    dt = mybir.dt
    AF = mybir.ActivationFunctionType
    OP = mybir.AluOpType
    AX = mybir.AxisListType
    f32 = dt.float32
    f32r = dt.float32r
    bf16 = dt.bfloat16

    nc = bacc.Bacc("TRN2", target_bir_lowering=False, debug=False,
                   num_devices=NCORES)

    # ---------------- I/O ----------------
    d_ids = nc.dram_tensor("ids", [BL, 1], dt.int32, kind="ExternalInput").ap()
    d_hid = nc.dram_tensor("hid", [B, H], f32, kind="ExternalInput").ap()
    d_hid_loc = nc.dram_tensor("hid_loc", [BL, H], f32, kind="ExternalInput").ap()
    d_hid_sl = nc.dram_tensor("hid_sl", [B, HS], f32, kind="ExternalInput").ap()
    d_enc = nc.dram_tensor("enc", [TX, BL, TWOH], f32, kind="ExternalInput").ap()
    d_emb = nc.dram_tensor("embW", [KY, E], f32, kind="ExternalInput").ap()
    d_aW1 = nc.dram_tensor("aW1", [H, K3H], f32, kind="ExternalInput").ap()
    d_ab1T = nc.dram_tensor("ab1T", [P, H // P], f32, kind="ExternalInput").ap()
    d_aW2T = nc.dram_tensor("aW2T", [P, H // P], f32, kind="ExternalInput").ap()
    d_wih = nc.dram_tensor("wih_s", [3, HS, KIN], f32, kind="ExternalInput").ap()
    d_whh = nc.dram_tensor("whh_s", [3, HS, H], f32, kind="ExternalInput").ap()
    d_bihT = nc.dram_tensor("bihT", [HS, 3], f32, kind="ExternalInput").ap()
    d_bhhT = nc.dram_tensor("bhhT", [HS, 3], f32, kind="ExternalInput").ap()
    d_lsm = nc.dram_tensor("lsm_s", [VSH, K3H], f32, kind="ExternalInput").ap()
    d_lsmb = nc.dram_tensor("lsmb_s", [1, VSH], f32, kind="ExternalInput").ap()

    d_out = nc.dram_tensor("out_part", [B, VSH], f32, kind="ExternalOutput").ap()
    d_hnew = nc.dram_tensor("h_new", [B, H], f32, kind="ExternalOutput").ap()

    RG = [list(range(NCORES))]
    NKC_E = TWOH // P   # 16
    NKC_H = H // P      # 8
    NM = H // P         # 8
    NKC = K3H // P      # 24

    with tile.TileContext(nc) as tc:
        with tc.tile_pool(name="const", bufs=1) as const, \
             tc.tile_pool(name="persist", bufs=1) as persist, \
             tc.tile_pool(name="dram", bufs=1, space="DRAM") as dram:

            psum_t_cm = tc.tile_pool(name="psum_t", bufs=2, space="PSUM")
            psum_t = psum_t_cm.__enter__()

            ident = const.tile([P, P], f32)
            make_identity(nc, ident)
            ident_bf = const.tile([P, P], bf16)
            nc.vector.tensor_copy(out=ident_bf[:], in_=ident[:])

            ag1_in = dram.tile([BL, KIN], f32)
            ag1_out = dram.tile([B, KIN], f32, addr_space="Shared")
            ag2_in = dram.tile([HS, B], f32)
            ag2_out = dram.tile([NCORES, HS, B], f32, addr_space="Shared")
            ag3_in = dram.tile([B, 2], f32)
            ag3_out = dram.tile([NCORES, B, 2], f32, addr_space="Shared")

            def pe_T(dst, src, _unused=None, dtype=f32, pool=None):
                """transpose src [n_in, ncols<=128] -> dst [ncols, n_in]"""
                pool = pool or psum_t
                n_in = src.shape[0]
                ncols = src.shape[-1]
                pt = pool.tile([P, P], dtype, tag="pe_t")
                idn = ident_bf if dtype == bf16 else ident
                nc.tensor.transpose(pt[:ncols, :n_in], src, idn[:n_in, :n_in])
                nc.any.tensor_copy(out=dst, in_=pt[:ncols, :n_in])

            # ======== phase 0: small weight prep (persistent tensors) ========
            hiddenT = persist.tile([P, NKC_H, B], f32r)
            hidT_loc = persist.tile([P, NKC_H, BL], f32r)
            hidT_sl = persist.tile([HS, B], f32)
            wihT = persist.tile([P, KIN // P, 3, HS], f32r)
            whhT = persist.tile([P, NKC_H, 3, HS], f32r)
            ab1T = persist.tile([P, NM], f32)
            aW2T = persist.tile([P, NM], f32)
            aW2Tr = persist.tile([P, NM], f32r)
            bihT = persist.tile([HS, 3], f32)
            bhhT = persist.tile([HS, 3], f32)
            brzT = persist.tile([HS, 3], f32)
            ones1f = persist.tile([1, B], f32)
            ones1 = persist.tile([1, B], f32r)
            ids_sb = persist.tile([BL, 1], dt.int32)
            emb_sb = persist.tile([BL, E], f32)

            with tc.tile_pool(name="prep", bufs=2) as prep:
                hid_nat = prep.tile([B, H], f32, tag="hidnat")
                nc.sync.dma_start(out=hid_nat[:], in_=d_hid)
                for kc in range(NKC_H):
                    pe_T(hiddenT[:, kc, :], hid_nat[:, kc * P:(kc + 1) * P], B)
                hid_loc_nat = prep.tile([BL, H], f32, tag="hidloc")
                nc.sync.dma_start(out=hid_loc_nat[:], in_=d_hid_loc)
                for kc in range(NKC_H):
                    pe_T(hidT_loc[:, kc, :],
                         hid_loc_nat[:, kc * P:(kc + 1) * P], BL)
                hid_sl_nat = prep.tile([B, HS], f32, tag="hidsl")
                nc.sync.dma_start(out=hid_sl_nat[:], in_=d_hid_sl)
                pe_T(hidT_sl[:, :], hid_sl_nat[:, :], B)

                for g in range(3):
                    wg = prep.tile([HS, KIN], f32, tag="wihnat")
                    nc.sync.dma_start(out=wg[:], in_=d_wih[g])
                    for kc in range(KIN // P):
                        pe_T(wihT[:, kc, g, :], wg[:, kc * P:(kc + 1) * P], HS)
                    wg2 = prep.tile([HS, H], f32, tag="whhnat")
                    nc.sync.dma_start(out=wg2[:], in_=d_whh[g])
                    for kc in range(NKC_H):
                        pe_T(whhT[:, kc, g, :], wg2[:, kc * P:(kc + 1) * P], HS)

                nc.sync.dma_start(out=ab1T[:], in_=d_ab1T)
                nc.sync.dma_start(out=aW2T[:], in_=d_aW2T)
                nc.vector.tensor_copy(out=aW2Tr[:], in_=aW2T[:])
                nc.sync.dma_start(out=bihT[:], in_=d_bihT)
                nc.sync.dma_start(out=bhhT[:], in_=d_bhhT)
                nc.vector.tensor_tensor(out=brzT[:], in0=bihT[:], in1=bhhT[:],
                                        op=OP.add)
                nc.any.memset(ones1f[:], 1.0)
                nc.vector.tensor_copy(out=ones1[:], in_=ones1f[:])
                nc.sync.dma_start(out=ids_sb[:], in_=d_ids)
                nc.gpsimd.indirect_dma_start(
                    out=emb_sb[:], out_offset=None, in_=d_emb,
                    in_offset=bass.IndirectOffsetOnAxis(ap=ids_sb[:, :1], axis=0))

            # ======== phase 1: alignment + context (rows = b*128 + t) ========
            with tc.tile_pool(name="encT_p", bufs=1) as encT_p, \
                 tc.tile_pool(name="align", bufs=2) as align, \
                 tc.tile_pool(name="aw1T_p", bufs=1) as aw1T_p, \
                 tc.tile_pool(name="psum_a", bufs=1, space="PSUM") as psum_a, \
                 tc.tile_pool(name="attn", bufs=1) as attn, \
                 tc.tile_pool(name="psum_s", bufs=1, space="PSUM") as psum_s:

                ctxT = attn.tile([P, NKC_E, BL], f32)
                e_sb = attn.tile([1, ROWS], f32)
                ssum = attn.tile([1, BL], f32)
                encT = encT_p.tile([P, NKC_E, ROWS], f32r)
                for bl in range(BL):
                    enat = align.tile([P, TWOH], f32, tag="encnat")
                    nc.sync.dma_start(out=enat[:], in_=d_enc[:, bl, :])
                    for kc in range(NKC_E):
                        pe_T(encT[:, kc, bl * P:(bl + 1) * P],
                             enat[:, kc * P:(kc + 1) * P], P)

                score_ps = psum_s.tile([1, ROWS], f32)
                for m in range(NM):
                    aw1T_m = aw1T_p.tile([P, NKC, P], f32r, tag="aw1T")
                    anat = align.tile([P, K3H], f32, tag="aw1nat")
                    nc.sync.dma_start(out=anat[:], in_=d_aW1[m * P:(m + 1) * P, :])
                    for kc in range(NKC):
                        pe_T(aw1T_m[:, kc, :], anat[:, kc * P:(kc + 1) * P], P)

                    # A_part[h,b] = sum_k aW1T[k,h]*hiddenT_loc[k,b] (k<H)
                    ap_ps = psum_t.tile([P, BL], f32, tag="apart")
                    for kc in range(NKC_H):
                        nc.tensor.matmul(
                            ap_ps[:], aw1T_m[:, kc, :],
                            hidT_loc[:, kc, :],
                            start=(kc == 0), stop=(kc == NKC_H - 1))
                    a_ps = psum_a.tile([P, ROWS], f32, tag="a_acc")
                    for hf in range(2):  # 512-wide halves (fp32 N<=512)
                        cs = slice(hf * 512, (hf + 1) * 512)
                        for kc in range(NKC_E):
                            nc.tensor.matmul(
                                a_ps[:, cs],
                                aw1T_m[:, NKC_H + kc, :],
                                encT[:, kc, cs],
                                start=(kc == 0), stop=(kc == NKC_E - 1))
                    # += A_part broadcast over t (rows b-outer, t-inner)
                    ap_sb = align.tile([P, BL], f32, tag="ap_sb")
                    nc.scalar.copy(ap_sb[:], ap_ps[:])
                    a3 = a_ps[:].rearrange("p (b t) -> p b t", b=BL)
                    nc.vector.tensor_tensor(
                        out=a3, in0=a3,
                        in1=ap_sb[:].unsqueeze(2).broadcast_to([P, BL, TX]),
                        op=OP.add)
                    tanh_t = align.tile([P, ROWS], f32r, tag="tanh")
                    nc.scalar.activation(tanh_t[:], a_ps[:], AF.Tanh,
                                         bias=ab1T[:, m:m + 1])
                    for hf in range(2):
                        cs = slice(hf * 512, (hf + 1) * 512)
                        nc.tensor.matmul(
                            score_ps[:, cs], aW2Tr[:, m:m + 1],
                            tanh_t[:, cs],
                            start=(m == 0), stop=(m == NM - 1))

                # softmax-free context: e = exp(score), ctx = (e.enc)/sum(e)
                nc.scalar.activation(e_sb[:], score_ps[:], AF.Exp)
                e_bc = attn.tile([P, ROWS], f32)
                nc.gpsimd.partition_broadcast(e_bc[:], e_sb[:])
                nc.vector.reduce_sum(
                    ssum[:], e_sb[:].rearrange("o (b t) -> o b t", b=BL),
                    axis=AX.X)
                for kc in range(NKC_E):
                    tmp = align.tile([P, ROWS], f32, tag="ctmp")
                    nc.vector.tensor_tensor(
                        out=tmp[:], in0=encT[:, kc, :], in1=e_bc[:],
                        op=OP.mult)
                    nc.vector.reduce_sum(
                        ctxT[:, kc, :],
                        tmp[:].rearrange("p (b t) -> p b t", b=BL), axis=AX.X)
                rs = attn.tile([1, BL], f32)
                nc.vector.reciprocal(rs[:], ssum[:])
                rs_bc = attn.tile([P, BL], f32)
                nc.gpsimd.partition_broadcast(rs_bc[:], rs[:])
                nc.vector.tensor_tensor(
                    out=ctxT[:], in0=ctxT[:],
                    in1=rs_bc[:].unsqueeze(1).broadcast_to([P, NKC_E, BL]),
                    op=OP.mult)

                # rnn_in local [8, 2560] = [embed | context]
                ctx_nat = attn.tile([BL, TWOH], f32)
                for kc in range(NKC_E):
                    pe_T(ctx_nat[:, kc * P:(kc + 1) * P], ctxT[:, kc, :], BL)
                nc.sync.dma_start(out=ag1_in[:, :E], in_=emb_sb[:])
                nc.sync.dma_start(out=ag1_in[:, E:], in_=ctx_nat[:])
                nc.gpsimd.collective_compute(
                    "AllGather", OP.bypass, replica_groups=RG,
                    ins=[ag1_in.opt()], outs=[ag1_out.opt()])

            # ======== phase 2: GRU (gate-sharded) ========
            late_cm = tc.tile_pool(name="late", bufs=1)
            late = late_cm.__enter__()
            rnnT = late.tile([P, KIN // P, B], f32r)
            hT_all = late.tile([P, NCORES, B], f32)
            xT = late.tile([P, NKC, B], bf16)
            with tc.tile_pool(name="gru", bufs=1) as gru, \
                 tc.tile_pool(name="psum_g", bufs=1, space="PSUM") as psum_g:
                rnn_nat = gru.tile([B, KIN], f32, tag="rnn_nat")
                nc.sync.dma_start(out=rnn_nat[:], in_=ag1_out[:])
                for kc in range(KIN // P):
                    pe_T(rnnT[:, kc, :], rnn_nat[:, kc * P:(kc + 1) * P], B)

                gates = []
                for g in (0, 1):  # r, z: gi+gh accumulate together
                    gp = psum_g.tile([HS, B], f32, tag=f"g{g}")
                    n_mm = KIN // P + NKC_H
                    i = 0
                    for kc in range(KIN // P):
                        nc.tensor.matmul(gp[:], wihT[:, kc, g, :],
                                         rnnT[:, kc, :],
                                         start=(i == 0), stop=(i == n_mm - 1))
                        i += 1
                    for kc in range(NKC_H):
                        nc.tensor.matmul(gp[:], whhT[:, kc, g, :],
                                         hiddenT[:, kc, :],
                                         start=(i == 0), stop=(i == n_mm - 1))
                        i += 1
                    gs = gru.tile([HS, B], f32, tag=f"gs{g}")
                    nc.scalar.activation(gs[:], gp[:], AF.Sigmoid,
                                         bias=brzT[:, g:g + 1])
                    gates.append(gs)
                r_sb, z_sb = gates
                in_ps = psum_g.tile([HS, B], f32, tag="in")
                for kc in range(KIN // P):
                    nc.tensor.matmul(in_ps[:], wihT[:, kc, 2, :],
                                     rnnT[:, kc, :],
                                     start=(kc == 0), stop=(kc == KIN // P - 1))
                hn_ps = psum_g.tile([HS, B], f32, tag="hn")
                for kc in range(NKC_H):
                    nc.tensor.matmul(hn_ps[:], whhT[:, kc, 2, :],
                                     hiddenT[:, kc, :],
                                     start=(kc == 0), stop=(kc == NKC_H - 1))
                hn_sb = gru.tile([HS, B], f32, tag="hnsb")
                nc.scalar.activation(hn_sb[:], hn_ps[:], AF.Identity,
                                     bias=bhhT[:, 2:3])
                t1 = gru.tile([HS, B], f32, tag="t1")
                nc.vector.tensor_tensor(out=t1[:], in0=r_sb[:], in1=hn_sb[:],
                                        op=OP.mult)
                nc.vector.tensor_tensor(out=t1[:], in0=t1[:], in1=in_ps[:],
                                        op=OP.add)
                n_sb = gru.tile([HS, B], f32, tag="nsb")
                nc.scalar.activation(n_sb[:], t1[:], AF.Tanh, bias=bihT[:, 2:3])
                hmn = gru.tile([HS, B], f32, tag="hmn")
                nc.vector.tensor_tensor(out=hmn[:], in0=hidT_sl[:], in1=n_sb[:],
                                        op=OP.subtract)
                nc.vector.tensor_tensor(out=hmn[:], in0=hmn[:], in1=z_sb[:],
                                        op=OP.mult)
                hnewT_s = gru.tile([HS, B], f32, tag="hnewT")
                nc.vector.tensor_tensor(out=hnewT_s[:], in0=n_sb[:],
                                        in1=hmn[:], op=OP.add)
                nc.sync.dma_start(out=ag2_in[:], in_=hnewT_s[:])
                nc.gpsimd.collective_compute(
                    "AllGather", OP.bypass, replica_groups=RG,
                    ins=[ag2_in.opt()], outs=[ag2_out.opt()])

                nc.sync.dma_start(out=hT_all[:],
                                  in_=ag2_out.rearrange("c h b -> h c b"))
                hnew_nat = gru.tile([B, H], f32, tag="hnew_nat")
                for c in range(NCORES):
                    pe_T(hnew_nat[:, c * P:(c + 1) * P], hT_all[:, c, :], B)
                nc.sync.dma_start(out=d_hnew, in_=hnew_nat[:])

                # XT (bf16): [h_newT | contextT(full batch)]
                for c in range(NCORES):
                    nc.vector.tensor_copy(out=xT[:, c, :], in_=hT_all[:, c, :])
                for j in range(NKC_E):
                    nc.vector.tensor_copy(out=xT[:, NCORES + j, :],
                                          in_=rnnT[:, E // P + j, :])

            psum_t_cm.__exit__(None, None, None)
            # ======== phase 3: LM head (vocab shard) + log-softmax ========
            logits = late.tile([B, VSH], f32)
            with tc.tile_pool(name="lm", bufs=2) as lm, \
                 tc.tile_pool(name="psum_l", bufs=2, space="PSUM") as psum_l, \
                 tc.tile_pool(name="psum_lt", bufs=4, space="PSUM") as psum_lt:
                GW = 256
                NG = VSH // GW  # 16
                lsmb1f = late.tile([1, VSH], f32)
                nc.sync.dma_start(out=lsmb1f[:], in_=d_lsmb)
                lsmb1 = late.tile([1, VSH], f32r)
                nc.vector.tensor_copy(out=lsmb1[:], in_=lsmb1f[:])
                for g in range(NG):
                    lsmT_g = lm.tile([P, NKC, GW], bf16, tag="lmT")
                    for vc in range(GW // P):
                        nat = lm.tile([P, K3H], bf16, tag="lmnat")
                        r0 = g * GW + vc * P
                        nc.gpsimd.dma_start(out=nat[:], in_=d_lsm[r0:r0 + P, :])
                        for kc in range(NKC):
                            pt = psum_lt.tile([P, P], bf16, tag="lt_ps")
                            nc.tensor.transpose(
                                pt[:], nat[:, kc * P:(kc + 1) * P], ident_bf[:])
                            nc.any.tensor_copy(
                                out=lsmT_g[:, kc, vc * P:(vc + 1) * P],
                                in_=pt[:])
                    lp = psum_l.tile([B, GW], f32, tag="lps")
                    for kc in range(NKC):
                        nc.tensor.matmul(lp[:], xT[:, kc, :], lsmT_g[:, kc, :],
                                         start=(kc == 0), stop=False)
                    nc.tensor.matmul(
                        lp[:], ones1[:],
                        lsmb1[:, g * GW:(g + 1) * GW],
                        start=False, stop=True)
                    nc.vector.tensor_copy(out=logits[:, g * GW:(g + 1) * GW],
                                          in_=lp[:])

                mx = late.tile([B, 1], f32)
                nc.vector.reduce_max(mx[:], logits[:], axis=AX.X)
                nmx = late.tile([B, 1], f32)
                nc.vector.tensor_scalar(out=nmx[:], in0=mx[:], scalar1=-1.0,
                                        scalar2=None, op0=OP.mult)
                se_parts = late.tile([B, NG], f32)
                for g in range(NG):
                    esc = lm.tile([B, GW], f32, tag="esc")
                    nc.scalar.activation(esc[:], logits[:, g * GW:(g + 1) * GW],
                                         AF.Exp, bias=nmx[:],
                                         accum_out=se_parts[:, g:g + 1])
                se = late.tile([B, 1], f32)
                nc.vector.reduce_sum(se[:], se_parts[:], axis=AX.X)
                stats = late.tile([B, 2], f32)
                nc.vector.tensor_copy(out=stats[:, 0:1], in_=mx[:])
                nc.vector.tensor_copy(out=stats[:, 1:2], in_=se[:])
                nc.sync.dma_start(out=ag3_in[:], in_=stats[:])
                nc.gpsimd.collective_compute(
                    "AllGather", OP.bypass, replica_groups=RG,
                    ins=[ag3_in.opt()], outs=[ag3_out.opt()])

                st_all = late.tile([B, NCORES, 2], f32)
                nc.sync.dma_start(out=st_all[:],
                                  in_=ag3_out.rearrange("c b s -> b c s"))
                gm = late.tile([B, 1], f32)
                nc.vector.reduce_max(gm[:], st_all[:, :, 0], axis=AX.X)
                md = late.tile([B, NCORES], f32)
                nc.vector.tensor_tensor(out=md[:], in0=st_all[:, :, 0],
                                        in1=gm[:].broadcast_to([B, NCORES]),
                                        op=OP.subtract)
                emd = late.tile([B, NCORES], f32)
                nc.scalar.activation(emd[:], md[:], AF.Exp)
                nc.vector.tensor_tensor(out=emd[:], in0=emd[:],
                                        in1=st_all[:, :, 1], op=OP.mult)
                gs_ = late.tile([B, 1], f32)
                nc.vector.reduce_sum(gs_[:], emd[:], axis=AX.X)
                lns = late.tile([B, 1], f32)
                nc.scalar.activation(lns[:], gs_[:], AF.Ln)
                nshift = late.tile([B, 1], f32)
                nc.vector.tensor_tensor(out=nshift[:], in0=gm[:], in1=lns[:],
                                        op=OP.add)
                nc.vector.tensor_scalar(out=nshift[:], in0=nshift[:],
                                        scalar1=-1.0, scalar2=None, op0=OP.mult)
                for g in range(NG):
                    osb = lm.tile([B, GW], f32, tag="osb")
                    nc.scalar.activation(osb[:],
                                         logits[:, g * GW:(g + 1) * GW],
                                         AF.Identity, bias=nshift[:])
                    nc.sync.dma_start(out=d_out[:, g * GW:(g + 1) * GW],
                                      in_=osb[:])

            late_cm.__exit__(None, None, None)

    nc.compile()
    return nc


def _shard_inputs(inputs):
    ids = np.asarray(inputs["input"]).astype(np.int32).reshape(B, 1)
    hid = np.ascontiguousarray(np.asarray(inputs["hidden"], dtype=np.float32))
    enc = np.ascontiguousarray(np.asarray(inputs["enc_output"], dtype=np.float32))
    embW = np.ascontiguousarray(np.asarray(inputs["emb_W"], dtype=np.float32))
    aW1 = np.ascontiguousarray(np.asarray(inputs["aW1"], dtype=np.float32))
    ab1 = np.asarray(inputs["ab1"], dtype=np.float32)
    aW2 = np.asarray(inputs["aW2"], dtype=np.float32)
    Wih = np.asarray(inputs["Wih"], dtype=np.float32).reshape(3, H, KIN)
    Whh = np.asarray(inputs["Whh"], dtype=np.float32).reshape(3, H, H)
    bih = np.asarray(inputs["bih"], dtype=np.float32).reshape(3, H)
    bhh = np.asarray(inputs["bhh"], dtype=np.float32).reshape(3, H)
    lsm_W = np.asarray(inputs["lsm_W"], dtype=np.float32)
    lsm_b = np.asarray(inputs["lsm_b"], dtype=np.float32)

    ab1T = np.ascontiguousarray(ab1.reshape(H // P, P).T)      # [128, 8]
    aW2T = np.ascontiguousarray(aW2.reshape(H // P, P).T)      # [128, 8]

    in_maps = []
    for c in range(NCORES):
        bsl = slice(c * BL, (c + 1) * BL)
        hsl = slice(c * HS, (c + 1) * HS)
        vlo = c * VSH
        if vlo + VSH <= KY:
            lsm_s = np.ascontiguousarray(lsm_W[vlo:vlo + VSH])
            lsmb_s = np.ascontiguousarray(lsm_b[vlo:vlo + VSH])
        else:
            nreal = max(0, KY - vlo)
            lsm_s = np.zeros((VSH, K3H), np.float32)
            lsm_s[:nreal] = lsm_W[vlo:vlo + nreal]
            lsmb_s = np.full((VSH,), -1.0e4, np.float32)
            lsmb_s[:nreal] = lsm_b[vlo:vlo + nreal]
        in_maps.append({
            "ids": np.ascontiguousarray(ids[bsl]),
            "hid": hid,
            "hid_loc": np.ascontiguousarray(hid[bsl]),
            "hid_sl": np.ascontiguousarray(hid[:, hsl]),
            "enc": np.ascontiguousarray(enc[:, bsl, :]),
            "embW": embW,
            "aW1": aW1,
            "ab1T": ab1T,
            "aW2T": aW2T,
            "wih_s": np.ascontiguousarray(Wih[:, hsl, :]),
            "whh_s": np.ascontiguousarray(Whh[:, hsl, :]),
            "bihT": np.ascontiguousarray(bih[:, hsl].T),
            "bhhT": np.ascontiguousarray(bhh[:, hsl].T),
            "lsm_s": lsm_s,
            "lsmb_s": np.ascontiguousarray(lsmb_s.reshape(1, VSH)),
        })
    return in_maps


def kernel(**inputs):
    from concourse.bass_utils import run_bass_kernel_spmd
    if "nc" not in _CACHE:
        _CACHE["nc"] = _program()
    nc = _CACHE["nc"]
    in_maps = _shard_inputs(inputs)
    res = run_bass_kernel_spmd(nc, in_maps, core_ids=list(range(NCORES)),
                               **_CACHE.get("run_kwargs", {}))
    _CACHE["last_results"] = res
    outs = res.results
    out_full = np.concatenate(
        [outs[c]["out_part"] for c in range(NCORES)], axis=1)[:, :KY]
    h_new = outs[0]["h_new"]
    return out_full.astype(np.float32), h_new.astype(np.float32)
